# revision 22
# baseline (speedup 1.0000x reference)
"""Trainium2 Bass kernel for nn_AxonalConnections.

Computes, per (batch b, patch n):
    out[t]  = sum_s sp[b,n,s] * W_dyn[b,n,t,s]          (batched matvec, distinct weights)
    out_n   = LayerNorm_T(out) * gamma + beta
    w       = softmax(out_n / TEMP)
    final   = w * (gates[n] * sum_s sp[b,n,s] + biases[n])
    fold -> [B, 256, 256]

Strategy: 8-way shard over (batch b, patch-half). Each core handles 128 patches.
On-chip layout keyed on partition = patch index n:
  - W streamed in [128, TB, 256] blocks (contiguous 16KB/partition DMAs)
  - fused dot product per target pixel t via DVE tensor_tensor_reduce
    (accum_out), so no transposes and no operand broadcasts are needed
  - LayerNorm + temperature softmax epilogue on DVE/ACT over [128, 256]
Unfold/fold and shard assembly are numpy index remaps done host-side.
"""

import sys

for _p in ("/opt/trn_rl_repo",):
    if _p not in sys.path:
        sys.path.insert(0, _p)

import numpy as np

import concourse.bass as bass
import concourse.bacc as bacc
import concourse.tile as tile
from concourse import mybir
from concourse import bass_utils

# Problem constants (hardcoded per contract)
B = 4
GRID = 256
PATCH = 16
PH = GRID // PATCH          # 16 patches per side
N = PH * PH                 # 256 patches
S = PATCH * PATCH           # 256 source pixels per patch
T = 256                     # 256 target pixels per patch
TEMP = 0.1
LN_EPS = 1e-5

NCORES = 8
P = 128                     # patches per core (= SBUF partitions)
TB = 16                     # target rows per W DMA block (16KB/partition/DMA)
NBLK = T // TB
JD = 5                      # j's per block reduced on DVE (rest on ScalarE)
POOL_BLOCKS = frozenset({3, 7, 11, 15})  # blocks whose mult runs on GPSIMD

F32 = mybir.dt.float32

_NC_CACHE = {}


def _build_nc():
    # Bacc (not raw Bass): its compile() runs generate_event_semaphores,
    # which splits multi-sem waits into EventSemaphore instructions — the
    # TRN2 "at most 1 wait per instruction" legalization walrus requires.
    nc = bacc.Bacc("TRN2")
    w = nc.dram_tensor("w", [P, T, S], F32, kind="ExternalInput")
    sp = nc.dram_tensor("sp", [P, S], F32, kind="ExternalInput")
    # packed per-core params: [gamma/TEMP (256) | beta/TEMP (256) | gate | bias]
    prm = nc.dram_tensor("prm", [P, 2 * T + 2], F32, kind="ExternalInput")
    outd = nc.dram_tensor("out", [P, T], F32, kind="ExternalOutput")

    Alu = mybir.AluOpType
    Act = mybir.ActivationFunctionType
    Ax = mybir.AxisListType

    with tile.TileContext(nc) as tc:
        with (
            tc.tile_pool(name="wpool", bufs=4) as wpool,
            tc.tile_pool(name="ppool", bufs=4) as ppool,
            tc.tile_pool(name="apool", bufs=2) as apool,
            tc.tile_pool(name="sing", bufs=1) as sing,
            tc.tile_pool(name="small", bufs=2) as small,
        ):
            sp_t = sing.tile([P, S], F32)
            nc.sync.dma_start(out=sp_t, in_=sp[:, :])
            prm_t = sing.tile([P, 2 * T + 2], F32)
            nc.sync.dma_start(out=prm_t, in_=prm[:, :])
            gmt_t = prm_t[:, 0:T]
            bft_t = prm_t[:, T : 2 * T]
            gat_t = prm_t[:, 2 * T : 2 * T + 1]
            bia_t = prm_t[:, 2 * T + 1 : 2 * T + 2]
            eps_t = sing.tile([P, 1], F32)
            nc.vector.memset(eps_t, LN_EPS)

            outm = sing.tile([P, T], F32)     # raw matvec results (n, t)

            # Plain TensorTensor ops only survive walrus codegen with <=1 sem
            # wait, so absorb each input DMA's completion into non-TT DVE ops
            # up-front: spsum reads sp_t, the touch reads prm_t.
            spsum = small.tile([P, 1], F32)
            nc.vector.tensor_reduce(out=spsum, in_=sp_t, axis=Ax.X, op=Alu.add)
            touch = small.tile([P, 1], F32)
            nc.vector.tensor_scalar_mul(touch, gmt_t[:, 0:1], 1.0)

            # sp replicated TB times -> per-block elementwise multiply needs
            # no broadcast APs. Copies run on GPSIMD (keeps DVE free; also
            # absorbs sp's DMA wait on the Pool proc).
            sp_rep = sing.tile([P, TB * S], F32)
            for r in range(TB):
                nc.gpsimd.tensor_copy(out=sp_rep[:, r * S : (r + 1) * S], in_=sp_t)

            # ---- main pass: stream W; mult on DVE; per-t reduction split
            # between DVE (batched 3D reduce) and ScalarE (activation accum) --
            for tb in range(NBLK):
                wt = wpool.tile([P, TB * S], F32)
                nc.sync.dma_start(
                    out=wt.rearrange("p (a b) -> p a b", a=TB),
                    in_=w[:, tb * TB : (tb + 1) * TB, :])
                prod = ppool.tile([P, TB * S], F32)
                meng = nc.gpsimd if tb in POOL_BLOCKS else nc.vector
                meng.tensor_tensor(out=prod, in0=wt, in1=sp_rep, op=Alu.mult)
                prod3 = prod.rearrange("p (a b) -> p a b", a=TB)
                if JD > 0:
                    nc.vector.tensor_reduce(
                        out=outm[:, tb * TB : tb * TB + JD],
                        in_=prod3[:, 0:JD, :], axis=Ax.X, op=Alu.add)
                for j in range(JD, TB):
                    t = tb * TB + j
                    act_o = apool.tile([P, S], F32)
                    nc.scalar.activation(
                        out=act_o, in_=prod3[:, j, :], func=Act.Copy,
                        scale=1.0, accum_out=outm[:, t : t + 1])

            # ---- LayerNorm over t ----
            stats = small.tile([P, 6], F32)
            nc.vector.bn_stats(out=stats, in_=outm)
            mv = small.tile([P, 2], F32)
            nc.vector.bn_aggr(out=mv, in_=stats)
            std = small.tile([P, 1], F32)
            nc.scalar.activation(out=std, in_=mv[:, 1:2], func=Act.Sqrt,
                                 bias=eps_t, scale=1.0)
            rstd = small.tile([P, 1], F32)
            nc.vector.reciprocal(out=rstd, in_=std)
            xc = small.tile([P, T], F32)
            nc.vector.tensor_scalar(out=xc, in0=outm, scalar1=mv[:, 0:1],
                                    scalar2=None, op0=Alu.subtract)
            z1 = small.tile([P, T], F32)
            nc.vector.tensor_scalar_mul(z1, xc, rstd)
            z2 = small.tile([P, T], F32)
            nc.vector.tensor_mul(z2, z1, gmt_t)
            z3 = small.tile([P, T], F32)
            nc.vector.tensor_add(z3, z2, bft_t)

            # ---- temperature softmax over t (1/TEMP folded into gmt/bft) ----
            mx = small.tile([P, 1], F32)
            nc.vector.tensor_reduce(out=mx, in_=z3, axis=Ax.X, op=Alu.max)
            negmx = small.tile([P, 1], F32)
            nc.vector.tensor_scalar_mul(negmx, mx, -1.0)
            e = small.tile([P, T], F32)
            den = small.tile([P, 1], F32)
            nc.scalar.activation(out=e, in_=z3, func=Act.Exp, bias=negmx,
                                 scale=1.0, accum_out=den)

            # ---- per-patch scalar: gates * sum(sp) + biases, / denom ----
            scal = small.tile([P, 1], F32)
            nc.vector.tensor_mul(scal, gat_t, spsum)
            scal2 = small.tile([P, 1], F32)
            nc.vector.tensor_add(scal2, scal, bia_t)
            rden = small.tile([P, 1], F32)
            nc.vector.reciprocal(out=rden, in_=den)
            fac = small.tile([P, 1], F32)
            nc.vector.tensor_mul(fac, scal2, rden)
            fin = small.tile([P, T], F32)
            nc.vector.tensor_scalar_mul(fin, e, fac)

            nc.sync.dma_start(out=outd[:, :], in_=fin)
    nc.compile()
    return nc


def _get_nc():
    if "nc" not in _NC_CACHE:
        _NC_CACHE["nc"] = _build_nc()
    return _NC_CACHE["nc"]


def _make_in_maps(source_spikes, W_dyn, ln_gamma, ln_beta, gates, biases):
    source_spikes = np.asarray(source_spikes, dtype=np.float32)
    W_dyn = np.asarray(W_dyn, dtype=np.float32)
    ln_gamma = np.asarray(ln_gamma, dtype=np.float32)
    ln_beta = np.asarray(ln_beta, dtype=np.float32)
    gates = np.asarray(gates, dtype=np.float32)
    biases = np.asarray(biases, dtype=np.float32)

    # unfold (matches reference._unfold with kernel=stride=16)
    sp_unf = (
        source_spikes.reshape(B, PH, PATCH, PH, PATCH)
        .transpose(0, 1, 3, 2, 4)
        .reshape(B, N, S)
    )
    sp_unf = np.ascontiguousarray(sp_unf)

    in_maps = []
    for c in range(NCORES):
        b, h = divmod(c, NCORES // B)
        n0 = h * P
        prm = np.empty((P, 2 * T + 2), dtype=np.float32)
        prm[:, 0:T] = ln_gamma / TEMP
        prm[:, T : 2 * T] = ln_beta / TEMP
        prm[:, 2 * T] = gates[n0 : n0 + P]
        prm[:, 2 * T + 1] = biases[n0 : n0 + P]
        in_maps.append({
            "w": np.ascontiguousarray(W_dyn[b, n0 : n0 + P]),
            "sp": np.ascontiguousarray(sp_unf[b, n0 : n0 + P]),
            "prm": prm,
        })
    return in_maps


def _assemble(results):
    out_bnt = np.empty((B, N, T), dtype=np.float32)
    for c in range(NCORES):
        b, h = divmod(c, NCORES // B)
        n0 = h * P
        out_bnt[b, n0 : n0 + P] = results[c]["out"]
    # fold (matches reference._fold)
    return np.ascontiguousarray(
        out_bnt.reshape(B, PH, PH, PATCH, PATCH)
        .transpose(0, 1, 3, 2, 4)
        .reshape(B, GRID, GRID)
    )


def run_sharded(inputs: dict, trace: bool = False):
    """Run the SPMD bass kernel on 8 cores. Returns (output, BassKernelResults)."""
    in_maps = _make_in_maps(**inputs)
    nc = _get_nc()
    res = bass_utils.run_bass_kernel_spmd(nc, in_maps, list(range(NCORES)),
                                          trace=trace)
    return _assemble(res.results), res


def kernel(**inputs) -> np.ndarray:
    out, _ = run_sharded(inputs, trace=False)
    return out
